# revision 28
# baseline (speedup 1.0000x reference)
"""ContextAwareAttention Trainium2 kernel (v3).

Strategy (sized for the TimelineSim cost model):
  - Data-parallel over batch: B=128 -> 16 batches/core x 8 cores; SBATCH=2
    batches per superbatch ("SB") iteration.
  - fp8e4m3 DoubleRow matmuls (0.5 cyc/row, 2x contraction per instruction)
    for q/k/v projections, Wo, and Wc1 (3-group Dekker split for accuracy).
    Weights are pre-scaled by powers of two into fp8 range; scales cancel
    exactly (exp scale / denominator ones value / output copy scale).
  - bf16 on the element-wise engines (DVE 2x perf modes).
  - Softmax denominator via a parallel ones @ P matmul reusing the scores
    PSUM bank; one [128,512] reciprocal feeds both heads' normalize.
  - mask * exp(rel-pos-bias) premultiplied on host into one bf16 table;
    single fused P multiply per (c,j).
  - DMAs spread across SP/DVE/Act hardware queues and merged into slabs so
    no single DMA queue paces the kernel.
  - LayerNorm: mu via DoubleRow matmul straight from fp8 context and the
    Wc1 row-sums; var from fp8 squares; rstd = exp(-0.5*ln(512*var+eps'));
    the activation-table set is pinned so there are zero table reloads.
  - ctx2 (Wc2) and out1 (Wo) accumulate into one PSUM tile per (chunk, j).
"""

import math

import numpy as np
import ml_dtypes

import concourse.bass as bass  # noqa: F401
import concourse.mybir as mybir
import concourse.tile as tile
from concourse import bacc
from concourse.bass_utils import run_bass_kernel_spmd

B, N, DIM, H, D = 128, 256, 512, 8, 64
N_CORES = 8
BPC = B // N_CORES            # 16
SBATCH = 2
NSUPER = BPC // SBATCH        # 8
SCALE = D ** -0.5
LN_EPS = 1e-5
F32 = mybir.dt.float32
F32R = mybir.dt.float32r
BF16 = mybir.dt.bfloat16
F8 = mybir.dt.float8e4
NW = SBATCH * N               # 512
NP8 = np.dtype(ml_dtypes.float8_e4m3)
NPBF = np.dtype(ml_dtypes.bfloat16)

AF = mybir.ActivationFunctionType
ALU = mybir.AluOpType
DR = mybir.MatmulPerfMode.DoubleRow

OT_K = 5   # ot is stored as 2^OT_K * O/denom (fp8 range health)


def _emit(nc, tc, io, n_super, ks):
    (x8d, c8d, clod, mbd, wq, wk, wv, wo, w1a, w1b, w1c, w1s, wc2, lngd, lnbd,
     bc1d, bocbd, outT) = io
    kq, kk, kv, kwo, kc1, ksum, guni = ks
    g = OT_K + kwo

    def mm(out, lhsT, rhs, start, stop, perf_mode=None):
        nc.tensor.matmul(out, lhsT, rhs, start=start, stop=stop,
                         perf_mode=perf_mode)

    with (
        nc.allow_low_precision(reason="fp8/bf16 design, verified vs oracle"),
        tc.tile_pool(name="consts", bufs=1) as consts,
        tc.tile_pool(name="io", bufs=3) as iop,
        tc.tile_pool(name="mb", bufs=3) as mbp,
        tc.tile_pool(name="work", bufs=2) as work,
        tc.tile_pool(name="pp", bufs=3) as ppool,
        tc.tile_pool(name="rows", bufs=2) as rows,
        tc.tile_pool(name="psum", bufs=2, space="PSUM") as psum,
    ):
        # ---- compile-time constants (no DMA) ----
        onecol = consts.tile([1, 128], BF16, name="onecol")
        nc.vector.memset(onecol, 1.0)
        # scaled so that ot = oo * (1/sbc) = 2^OT_K * O / denom exactly
        ones128 = consts.tile([128, 128], BF16, name="ones128")
        nc.vector.memset(ones128, 2.0 ** (kv - OT_K))
        colones = consts.tile([128, 1], BF16, name="colones")
        nc.vector.memset(colones, 1.0)
        eps512 = consts.tile([1, 1], F32, name="eps512")
        nc.vector.memset(eps512, DIM * LN_EPS)

        # ---- DMA'd constants: q/k/v weights on SP (interleaved with the
        # first superbatch's inputs below); the rest on the Act queue ----
        w8 = {}
        for nm, src in (("wq", wq), ("wk", wk), ("wv", wv)):
            t = consts.tile([128, 4, 512], F8, name=f"w_{nm}")
            nc.sync.dma_start(out=t,
                              in_=src.rearrange("(kc p) f -> p kc f", p=128))
            w8[nm] = t
        for nm, src in (("wo", wo), ("w1a", w1a), ("w1b", w1b), ("w1c", w1c)):
            t = consts.tile([128, 4, 512], F8, name=f"w_{nm}")
            nc.scalar.dma_start(out=t,
                                in_=src.rearrange("(kc p) f -> p kc f", p=128))
            w8[nm] = t
        w1st = consts.tile([128, 4, 1], BF16, name="w1s")
        nc.scalar.dma_start(out=w1st,
                            in_=w1s.rearrange("(kc p) f -> p kc f", p=128))
        wc2t = consts.tile([128, 4, 512], BF16, name="wc2")
        nc.scalar.dma_start(out=wc2t,
                            in_=wc2.rearrange("(kc p) f -> p kc f", p=128))
        lngc = consts.tile([128, 4], F32, name="lngc")   # ln_g * sqrt(512)
        nc.scalar.dma_start(out=lngc, in_=lngd.rearrange("(c p) -> p c", p=128))
        lnbc = consts.tile([128, 4], F32, name="lnbc")
        nc.scalar.dma_start(out=lnbc, in_=lnbd.rearrange("(c p) -> p c", p=128))
        bc1c = consts.tile([128, 4], F32, name="bc1c")
        nc.scalar.dma_start(out=bc1c, in_=bc1d.rearrange("(c p) -> p c", p=128))
        bocbc = consts.tile([128, 4], F32, name="bocbc")
        nc.scalar.dma_start(out=bocbc, in_=bocbd.rearrange("(c p) -> p c", p=128))

        carry = []
        carry2 = []
        for sb in range(n_super):
            b0 = sb * SBATCH
            # ---- input DMAs: one slab per tensor on the SP queue ----
            xt8 = iop.tile([128, 4, SBATCH, 256], F8, name="xt8")
            ct8 = iop.tile([128, 4, SBATCH, 256], F8, name="ct8")
            clo = iop.tile([128, 4, SBATCH, 256], F8, name="clo")
            nc.sync.dma_start(out=xt8, in_=x8d[sb])
            nc.sync.dma_start(out=ct8, in_=c8d[sb])
            nc.sync.dma_start(out=clo, in_=clod[sb])
            # mask*bias tables: one slab per j, on the DVE / Act queues
            mbt = {}
            for j in range(SBATCH):
                t = mbp.tile([128, 4, 2, 2, 256], BF16, name=f"mbt{j}")
                eng = nc.sync if j == 0 else nc.scalar
                eng.dma_start(
                    out=t, in_=mbd[b0 + j].rearrange(
                        "c mc h2 p n -> p c mc h2 n"))
                mbt[j] = t
            for fn in carry:
                fn()
            carry.clear()

            # ---- projections: fp8 DoubleRow ----
            qt = work.tile([128, 4, SBATCH, 256], BF16, name="qt")
            kt = work.tile([128, 4, SBATCH, 256], BF16, name="kt")
            for c in range(4):
                ps = psum.tile([128, NW], F32, tag="g", bufs=2)
                for i in range(2):
                    mm(ps, w8["wq"][:, 2 * i:2 * i + 2, c * 128:(c + 1) * 128],
                       xt8[:, 2 * i:2 * i + 2], start=i == 0, stop=i == 1,
                       perf_mode=DR)
                nc.scalar.copy(
                    out=qt[:, c].rearrange("p j n -> p (j n)"), in_=ps)
            for c in range(4):
                ps = psum.tile([128, NW], F32, tag="g", bufs=2)
                for i in range(2):
                    mm(ps, w8["wk"][:, 2 * i:2 * i + 2, c * 128:(c + 1) * 128],
                       ct8[:, 2 * i:2 * i + 2], start=i == 0, stop=i == 1,
                       perf_mode=DR)
                nc.scalar.copy(
                    out=kt[:, c].rearrange("p j n -> p (j n)"), in_=ps)
            # v token-major
            vt = work.tile([128, SBATCH, 2, 512], BF16, name="vt")
            for j in range(SBATCH):
                for mc in range(2):
                    ps = psum.tile([128, 512], F32, tag="g", bufs=2)
                    for i in range(2):
                        mm(ps, ct8[:, 2 * i:2 * i + 2, j, mc * 128:(mc + 1) * 128],
                           w8["wv"][:, 2 * i:2 * i + 2], start=i == 0, stop=i == 1,
                           perf_mode=DR)
                    nc.vector.tensor_copy(out=vt[:, j, mc, :], in_=ps)

            for fn in carry2:
                fn()
            carry2.clear()

            # ---- context branch: h = c @ Wc1 + bc1, 3-group fp8 Dekker ----
            # h*2^kc1 = c8@(W1A+W1B) + (32*dc)@W1C   (W1C = Wc1*2^kc1/32)
            ht = work.tile([128, 4, NW], BF16, name="ht")
            sqt = work.tile([128, 4, NW], F8, name="sqt")
            for c in range(4):
                ps = psum.tile([128, NW], F32, tag="g", bufs=2)
                cs = slice(c * 128, (c + 1) * 128)
                for i in range(2):
                    mm(ps, w8["w1a"][:, 2 * i:2 * i + 2, cs],
                       ct8[:, 2 * i:2 * i + 2], start=i == 0, stop=False,
                       perf_mode=DR)
                for i in range(2):
                    mm(ps, w8["w1b"][:, 2 * i:2 * i + 2, cs],
                       ct8[:, 2 * i:2 * i + 2], start=False, stop=False,
                       perf_mode=DR)
                for i in range(2):
                    mm(ps, w8["w1c"][:, 2 * i:2 * i + 2, cs],
                       clo[:, 2 * i:2 * i + 2], start=False, stop=i == 1,
                       perf_mode=DR)
                nc.scalar.activation(out=ht[:, c], in_=ps, func=AF.Identity,
                                     scale=2.0 ** (-kc1),
                                     bias=bc1c[:, c:c + 1])
                nc.gpsimd.tensor_mul(out=sqt[:, c], in0=ht[:, c], in1=ht[:, c])

            # ---- LN stats (closures; interleaved into attention) ----
            state = {}

            def ln_mu():
                # mu_sum via Wc1 row-sum weights applied to the fp8 context
                mu_ps = psum.tile([128, NW], F32, tag="g", bufs=2)
                for kc in range(4):
                    mm(mu_ps[0:1, :], w1st[:, kc, :],
                       ct8[:, kc].rearrange("p j n -> p (j n)"),
                       start=kc == 0, stop=kc == 3)
                mu_r = rows.tile([1, NW], F32, tag="r", bufs=6)
                nc.scalar.copy(out=mu_r, in_=mu_ps[0:1, :])
                state["mu_r"] = mu_r

            def ln_var():
                sq_ps = psum.tile([128, NW], F32, tag="g", bufs=2)
                for kc in range(4):
                    mm(sq_ps[0:1, :], colones, sqt[:, kc],
                       start=kc == 0, stop=kc == 3)
                mu_r = state["mu_r"]
                ms_r = rows.tile([1, NW], F32, tag="r", bufs=6)
                nc.vector.scalar_tensor_tensor(
                    out=ms_r, in0=mu_r, scalar=1.0 / DIM,
                    in1=mu_r, op0=ALU.mult, op1=ALU.mult)
                var_r = rows.tile([1, NW], F32, tag="r", bufs=6)
                nc.vector.tensor_sub(out=var_r, in0=sq_ps[0:1, :], in1=ms_r)
                ln_r = rows.tile([1, NW], F32, tag="r", bufs=6)
                nc.scalar.activation(out=ln_r, in_=var_r, func=AF.Ln,
                                     bias=eps512)
                a_r = rows.tile([1, NW], BF16, tag="r", bufs=6)
                nc.scalar.activation(out=a_r, in_=ln_r, func=AF.Exp,
                                     scale=-0.5)
                d_r = rows.tile([1, NW], BF16, tag="r", bufs=6)
                nc.vector.scalar_tensor_tensor(
                    out=d_r, in0=mu_r, scalar=-1.0 / DIM,
                    in1=a_r, op0=ALU.mult, op1=ALU.mult)
                ad_ps = psum.tile([128, 2, NW], F32, tag="s", bufs=2)
                mm(ad_ps[:, 0, :], onecol, a_r, start=True, stop=True)
                mm(ad_ps[:, 1, :], onecol, d_r, start=True, stop=True)
                ad_sb = work.tile([128, 2, NW], BF16, name="ad_sb")
                nc.vector.tensor_copy(out=ad_sb, in_=ad_ps)
                state["ad_sb"] = ad_sb

            # ---- attention + interleaved LN-normalize / ctx2+out1 ----
            ot = work.tile([128, 4, SBATCH, 256], F8, name="ot")
            res = iop.tile([128, 4, SBATCH, 256], BF16, name="res")

            def attn1(c, j):
                s_ps = psum.tile([128, 2, NW], F32, tag="s", bufs=2)
                for h2 in range(2):
                    p0 = 64 * h2
                    for mc in range(2):
                        mm(s_ps[:, h2, mc * 256:(mc + 1) * 256],
                           kt[p0:p0 + 64, c, j, mc * 128:(mc + 1) * 128],
                           qt[p0:p0 + 64, c, j], start=True, stop=True)
                pt = ppool.tile([128, 2, 2, 256], BF16, tag="p", name="pt",
                                bufs=6)
                nc.scalar.activation(
                    out=pt.rearrange("p mc h2 n -> p h2 mc n"),
                    in_=s_ps.rearrange("p h2 (mc n) -> p h2 mc n", mc=2),
                    func=AF.Exp, scale=2.0 ** (-(kq + kk)))
                nc.vector.tensor_mul(out=pt, in0=pt, in1=mbt[j][:, c])
                return s_ps, pt

            def attn2(c, j, s_ps, pt):
                # reuse the scores tile's first bank for the denominator
                sbc = s_ps[:, 0, :]
                for mc in range(2):
                    mm(sbc, ones128,
                       pt[:, mc].rearrange("p h n -> p (h n)"),
                       start=mc == 0, stop=mc == 1)
                oo = psum.tile([64, 2, 256], F32, tag="oo", bufs=2)
                for h2 in range(2):
                    hd = (2 * c + h2) * 64
                    for mc in range(2):
                        mm(oo[:, h2, :], vt[:, j, mc, hd:hd + 64],
                           pt[:, mc, h2, :], start=mc == 0, stop=mc == 1)
                rec_sb = ppool.tile([128, NW], F32, tag="rb", name="rec_sb",
                                    bufs=4)
                nc.vector.reciprocal(out=rec_sb, in_=sbc)
                for h2 in range(2):
                    nc.vector.tensor_mul(
                        out=ot[h2 * 64:(h2 + 1) * 64, c, j],
                        in0=oo[:, h2, :],
                        in1=rec_sb[h2 * 64:(h2 + 1) * 64,
                                   h2 * 256:(h2 + 1) * 256])

            def normalize(c):
                # rl = relu(((h*a + d)) * (g*sqrt(512)) + b), in place
                ad_sb = state["ad_sb"]
                nc.gpsimd.tensor_mul(out=ht[:, c], in0=ht[:, c],
                                     in1=ad_sb[:, 0, :])
                nc.gpsimd.tensor_add(out=ht[:, c], in0=ht[:, c],
                                     in1=ad_sb[:, 1, :])
                if guni is not None:
                    # relu(g*x) = g*max(x,0) for uniform g>0, b=0
                    nc.gpsimd.tensor_scalar(
                        out=ht[:, c], in0=ht[:, c], scalar1=0.0,
                        scalar2=guni * math.sqrt(DIM),
                        op0=ALU.max, op1=ALU.mult)
                else:
                    nc.scalar.activation(out=ht[:, c], in_=ht[:, c],
                                         func=AF.Relu,
                                         scale=lngc[:, c:c + 1],
                                         bias=lnbc[:, c:c + 1])

            def ctx2wo(j, ht=ht, ot=ot, res=res, b0=b0):
                co = psum.tile([128, 2, NW], F32, tag="s", bufs=2)
                for f in range(4):
                    dst = co[:, f // 2, (f % 2) * 256:(f % 2) * 256 + 256]
                    for kc in range(4):
                        mm(dst, wc2t[:, kc, f * 128:(f + 1) * 128],
                           ht[:, kc, j * 256:(j + 1) * 256],
                           start=kc == 0, stop=False)
                    for i in range(2):
                        mm(dst,
                           w8["wo"][:, 2 * i:2 * i + 2, f * 128:(f + 1) * 128],
                           ot[:, 2 * i:2 * i + 2, j, :],
                           start=False, stop=i == 1, perf_mode=DR)
                for f in range(4):
                    nc.scalar.activation(
                        out=res[:, f, j, :],
                        in_=co[:, f // 2, (f % 2) * 256:(f % 2) * 256 + 256],
                        func=AF.Identity, scale=2.0 ** (-g),
                        bias=bocbc[:, f:f + 1])
                carry.append(lambda j=j, res=res, b0=b0: nc.sync.dma_start(
                    out=outT[b0 + j].rearrange("(c p) n -> p c n", p=128),
                    in_=res[:, :, j, :]))

            # software-pipelined emission: stage2(n) always comes after
            # stage1(n+1) so no engine queue blocks on a not-yet-ready op
            p00 = attn1(0, 0)
            p10 = attn1(1, 0)
            attn2(0, 0, *p00)
            ln_mu()
            p20 = attn1(2, 0)
            attn2(1, 0, *p10)
            ln_var()
            p30 = attn1(3, 0)
            attn2(2, 0, *p20)
            normalize(0)
            normalize(1)
            p01 = attn1(0, 1)
            attn2(3, 0, *p30)
            normalize(2)
            normalize(3)
            p11 = attn1(1, 1)
            attn2(0, 1, *p01)
            ctx2wo(0)
            p21 = attn1(2, 1)
            attn2(1, 1, *p11)
            p31 = attn1(3, 1)
            attn2(2, 1, *p21)
            attn2(3, 1, *p31)
            carry2.append(lambda f=ctx2wo: f(1))
        for fn in carry2:
            fn()
        for fn in carry:
            fn()


def build(n_super, ks):
    # Pin the activation table: expose only natural_log_exp_and_others
    # (contains Exp/Ln/Relu/Identity/Copy/Square) to the act-table-load
    # insertion pass so it never flip-flops between sets.  Other entries are
    # emptied (not removed) to keep act_func_set_id indices valid.
    import concourse.bacc as bacc_mod
    from concourse.hw_specs import get_activation_tables as _gat

    def pinned_tables(arch):
        tabs = _gat(arch)
        return {name: (s if name == "natural_log_exp_and_others" else set())
                for name, s in tabs.items()}

    nc = bacc.Bacc("TRN2", target_bir_lowering=False, debug=False,
                   num_devices=N_CORES)
    dt = nc.dram_tensor
    io = (
        dt("x8", [NSUPER, 128, 4, SBATCH, N], F8,
           kind="ExternalInput").ap(),
        dt("c8", [NSUPER, 128, 4, SBATCH, N], F8,
           kind="ExternalInput").ap(),
        dt("clo", [NSUPER, 128, 4, SBATCH, N], F8,
           kind="ExternalInput").ap(),
        dt("mb", [BPC, 4, 2, 2, 128, N], BF16, kind="ExternalInput").ap(),
        dt("wq", [DIM, DIM], F8, kind="ExternalInput").ap(),
        dt("wk", [DIM, DIM], F8, kind="ExternalInput").ap(),
        dt("wv", [DIM, DIM], F8, kind="ExternalInput").ap(),
        dt("wo", [DIM, DIM], F8, kind="ExternalInput").ap(),
        dt("w1a", [DIM, DIM], F8, kind="ExternalInput").ap(),
        dt("w1b", [DIM, DIM], F8, kind="ExternalInput").ap(),
        dt("w1c", [DIM, DIM], F8, kind="ExternalInput").ap(),
        dt("w1s", [DIM, 1], BF16, kind="ExternalInput").ap(),
        dt("wc2", [DIM, DIM], BF16, kind="ExternalInput").ap(),
        dt("lng", [DIM], F32, kind="ExternalInput").ap(),
        dt("lnb", [DIM], F32, kind="ExternalInput").ap(),
        dt("bc1", [DIM], F32, kind="ExternalInput").ap(),
        dt("bocb", [DIM], F32, kind="ExternalInput").ap(),
        dt("outT", [BPC, DIM, N], BF16, kind="ExternalOutput").ap(),
    )
    with tile.TileContext(nc) as tc:
        _emit(nc, tc, io, n_super, ks)
    saved = bacc_mod.get_activation_tables
    bacc_mod.get_activation_tables = pinned_tables
    try:
        nc.compile()
    finally:
        bacc_mod.get_activation_tables = saved
    return nc


def _k_of(absmax):
    return int(math.floor(math.log2(120.0 / max(absmax, 1e-30))))


def prep_in_maps(x, context, mask, Wq, Wk, Wv, Wc1, bc1, ln_g, ln_b, Wc2, bc2,
                 Wo, bo, bias_table, rel_index):
    f = np.float32
    x = np.asarray(x, f)
    context = np.asarray(context, f)
    mask = np.asarray(mask)
    Wq = np.asarray(Wq, f) * SCALE
    Wk = np.asarray(Wk, f)
    Wv = np.asarray(Wv, f)
    Wo = np.asarray(Wo, f)
    Wc1 = np.asarray(Wc1, f)
    Wc2 = np.asarray(Wc2, f)

    kq = _k_of(np.abs(Wq).max())
    kk = _k_of(np.abs(Wk).max())
    kv = _k_of(np.abs(Wv).max())
    kwo = _k_of(np.abs(Wo).max())
    kc1 = _k_of(np.abs(Wc1).max())
    w1sum = Wc1.sum(axis=1, keepdims=True)
    ksum = 0
    lng_a = np.asarray(ln_g, f)
    lnb_a = np.asarray(ln_b, f)
    guni = (float(lng_a[0]) if np.all(lng_a == lng_a[0]) and float(lng_a[0]) > 0
            and np.all(lnb_a == 0.0) else None)
    ks = (kq, kk, kv, kwo, kc1, ksum, guni)
    g = OT_K + kwo

    xT = np.ascontiguousarray(
        x.reshape(N_CORES, BPC, N, DIM).transpose(0, 1, 3, 2))
    cT = np.ascontiguousarray(
        context.reshape(N_CORES, BPC, N, DIM).transpose(0, 1, 3, 2))
    x8 = xT.astype(NP8)
    c8 = cT.astype(NP8)
    clo = ((cT - c8.astype(f)) * 32.0).astype(NP8)

    def slab(a):
        # [cr, BPC, 512, 256] -> [cr, NSUPER, 128(p), 4(kc), SBATCH(j), 256]
        a = a.reshape(N_CORES, NSUPER, SBATCH, 4, 128, N)
        return np.ascontiguousarray(a.transpose(0, 1, 4, 3, 2, 5))

    x8 = slab(x8)
    c8 = slab(c8)
    clo = slab(clo)

    # mb[core, b, c, mc, h2, p, n] = maskT[b, m, n] * exp(bias)[h, m, n]
    expBT = np.exp(
        np.asarray(bias_table, f)[np.asarray(rel_index)].transpose(2, 1, 0))
    mT = mask.reshape(N_CORES, BPC, N, N).transpose(0, 1, 3, 2).astype(f)
    mbf = mT[:, :, None, :, :] * expBT[None, None, :, :, :]  # [cr,b,h,m,n]
    mbf = mbf.reshape(N_CORES, BPC, 4, 2, 2, 128, N).transpose(
        0, 1, 2, 4, 3, 5, 6)  # [cr, b, c, mc, h2, p, n]
    mb = np.ascontiguousarray(mbf).astype(NPBF)

    w1as = Wc1 * 2.0 ** kc1
    w1a = w1as.astype(NP8)
    w1b = (w1as - w1a.astype(f)).astype(NP8)
    w1c = (w1as / 32.0).astype(NP8)

    shared = dict(
        wq=np.ascontiguousarray(Wq * 2.0 ** kq).astype(NP8),
        wk=np.ascontiguousarray(Wk * 2.0 ** kk).astype(NP8),
        wv=np.ascontiguousarray(Wv * 2.0 ** kv).astype(NP8),
        wo=np.ascontiguousarray(Wo * 2.0 ** kwo).astype(NP8),
        w1a=np.ascontiguousarray(w1a),
        w1b=np.ascontiguousarray(w1b),
        w1c=np.ascontiguousarray(w1c),
        w1s=np.ascontiguousarray(w1sum).astype(NPBF),
        wc2=np.ascontiguousarray(Wc2 * 2.0 ** g).astype(NPBF),
        lng=np.ascontiguousarray(np.asarray(ln_g, f) * math.sqrt(DIM)),
        lnb=np.ascontiguousarray(np.asarray(ln_b, f)),
        bc1=np.ascontiguousarray(np.asarray(bc1, f)),
        bocb=np.ascontiguousarray(np.asarray(bo, f) + np.asarray(bc2, f)),
    )
    in_maps = [dict(x8=x8[c], c8=c8[c], clo=clo[c], mb=mb[c], **shared)
               for c in range(N_CORES)]
    return in_maps, ks


_nc_cache = {}


def _get_nc(n_super, ks):
    key = (n_super, ks)
    if key not in _nc_cache:
        _nc_cache[key] = build(n_super, ks)
    return _nc_cache[key]


def assemble_out(results):
    outT = np.stack([np.asarray(results[c]["outT"]).astype(np.float32)
                     for c in range(N_CORES)])
    return np.ascontiguousarray(
        outT.transpose(0, 1, 3, 2).reshape(B, N, DIM))


def kernel(**inputs):
    in_maps, ks = prep_in_maps(**inputs)
    nc = _get_nc(NSUPER, ks)
    res = run_bass_kernel_spmd(nc, in_maps, core_ids=list(range(N_CORES)))
    return assemble_out(res.results)


# revision 29
# speedup vs baseline: 1.0751x; 1.0751x over previous
"""ContextAwareAttention Trainium2 kernel (v3).

Strategy (sized for the TimelineSim cost model):
  - Data-parallel over batch: B=128 -> 16 batches/core x 8 cores; SBATCH=2
    batches per superbatch ("SB") iteration.
  - fp8e4m3 DoubleRow matmuls (0.5 cyc/row, 2x contraction per instruction)
    for q/k/v projections, Wo, and Wc1 (3-group Dekker split for accuracy).
    Weights are pre-scaled by powers of two into fp8 range; scales cancel
    exactly (exp scale / denominator ones value / output copy scale).
  - bf16 on the element-wise engines (DVE 2x perf modes).
  - Softmax denominator via a parallel ones @ P matmul reusing the scores
    PSUM bank; one [128,512] reciprocal feeds both heads' normalize.
  - mask * exp(rel-pos-bias) premultiplied on host into one bf16 table;
    single fused P multiply per (c,j).
  - DMAs spread across SP/DVE/Act hardware queues and merged into slabs so
    no single DMA queue paces the kernel.
  - LayerNorm: mu via DoubleRow matmul straight from fp8 context and the
    Wc1 row-sums; var from fp8 squares; rstd = exp(-0.5*ln(512*var+eps'));
    the activation-table set is pinned so there are zero table reloads.
  - ctx2 (Wc2) and out1 (Wo) accumulate into one PSUM tile per (chunk, j).
"""

import math

import numpy as np
import ml_dtypes

import concourse.bass as bass  # noqa: F401
import concourse.mybir as mybir
import concourse.tile as tile
from concourse import bacc
from concourse.bass_utils import run_bass_kernel_spmd

B, N, DIM, H, D = 128, 256, 512, 8, 64
N_CORES = 8
BPC = B // N_CORES            # 16
SBATCH = 2
NSUPER = BPC // SBATCH        # 8
SCALE = D ** -0.5
LN_EPS = 1e-5
F32 = mybir.dt.float32
F32R = mybir.dt.float32r
BF16 = mybir.dt.bfloat16
F8 = mybir.dt.float8e4
NW = SBATCH * N               # 512
NP8 = np.dtype(ml_dtypes.float8_e4m3)
NPBF = np.dtype(ml_dtypes.bfloat16)

AF = mybir.ActivationFunctionType
ALU = mybir.AluOpType
DR = mybir.MatmulPerfMode.DoubleRow

OT_K = 5   # ot is stored as 2^OT_K * O/denom (fp8 range health)


def _emit(nc, tc, io, n_super, ks):
    (x8d, c8d, clod, mbd, wq, wk, wv, wo, w1a, w1b, w1c, w1s, wc2, lngd, lnbd,
     bc1d, bocbd, outT) = io
    kq, kk, kv, kwo, kc1, ksum, guni = ks
    g = OT_K + kwo

    def mm(out, lhsT, rhs, start, stop, perf_mode=None):
        nc.tensor.matmul(out, lhsT, rhs, start=start, stop=stop,
                         perf_mode=perf_mode)

    with (
        nc.allow_low_precision(reason="fp8/bf16 design, verified vs oracle"),
        tc.tile_pool(name="consts", bufs=1) as consts,
        tc.tile_pool(name="io", bufs=3) as iop,
        tc.tile_pool(name="mb", bufs=3) as mbp,
        tc.tile_pool(name="work", bufs=2) as work,
        tc.tile_pool(name="pp", bufs=3) as ppool,
        tc.tile_pool(name="rows", bufs=2) as rows,
        tc.tile_pool(name="psum", bufs=2, space="PSUM") as psum,
    ):
        # ---- compile-time constants (no DMA) ----
        onecol = consts.tile([1, 128], BF16, name="onecol")
        nc.vector.memset(onecol, 1.0)
        # scaled so that ot = oo * (1/sbc) = 2^OT_K * O / denom exactly
        ones128 = consts.tile([128, 128], BF16, name="ones128")
        nc.vector.memset(ones128, 2.0 ** (kv - OT_K))
        colones = consts.tile([128, 1], BF16, name="colones")
        nc.vector.memset(colones, 1.0)
        eps512 = consts.tile([1, 1], F32, name="eps512")
        nc.vector.memset(eps512, DIM * LN_EPS)

        # ---- DMA'd constants: q/k/v weights on SP (interleaved with the
        # first superbatch's inputs below); the rest on the Act queue ----
        w8 = {}
        for nm, src in (("wq", wq), ("wk", wk), ("wv", wv)):
            t = consts.tile([128, 4, 512], F8, name=f"w_{nm}")
            nc.sync.dma_start(out=t,
                              in_=src.rearrange("(kc p) f -> p kc f", p=128))
            w8[nm] = t
        for nm, src in (("wo", wo), ("w1a", w1a), ("w1b", w1b), ("w1c", w1c)):
            t = consts.tile([128, 4, 512], F8, name=f"w_{nm}")
            nc.scalar.dma_start(out=t,
                                in_=src.rearrange("(kc p) f -> p kc f", p=128))
            w8[nm] = t
        w1st = consts.tile([128, 4, 1], BF16, name="w1s")
        nc.scalar.dma_start(out=w1st,
                            in_=w1s.rearrange("(kc p) f -> p kc f", p=128))
        wc2t = consts.tile([128, 4, 512], BF16, name="wc2")
        nc.scalar.dma_start(out=wc2t,
                            in_=wc2.rearrange("(kc p) f -> p kc f", p=128))
        lngc = consts.tile([128, 4], F32, name="lngc")   # ln_g * sqrt(512)
        nc.scalar.dma_start(out=lngc, in_=lngd.rearrange("(c p) -> p c", p=128))
        lnbc = consts.tile([128, 4], F32, name="lnbc")
        nc.scalar.dma_start(out=lnbc, in_=lnbd.rearrange("(c p) -> p c", p=128))
        bc1c = consts.tile([128, 4], F32, name="bc1c")
        nc.scalar.dma_start(out=bc1c, in_=bc1d.rearrange("(c p) -> p c", p=128))
        bocbc = consts.tile([128, 4], F32, name="bocbc")
        nc.scalar.dma_start(out=bocbc, in_=bocbd.rearrange("(c p) -> p c", p=128))

        carry = []
        carry2 = []
        for sb in range(n_super):
            b0 = sb * SBATCH
            # ---- input DMAs: one slab per tensor on the SP queue ----
            xt8 = iop.tile([128, 4, SBATCH, 256], F8, name="xt8")
            ct8 = iop.tile([128, 4, SBATCH, 256], F8, name="ct8")
            clo = iop.tile([128, 4, SBATCH, 256], F8, name="clo")
            nc.sync.dma_start(out=xt8, in_=x8d[sb])
            nc.sync.dma_start(out=ct8, in_=c8d[sb])
            nc.sync.dma_start(out=clo, in_=clod[sb])
            # mask*bias tables: one slab per j, on the DVE / Act queues
            mbt = {}
            for j in range(SBATCH):
                t = mbp.tile([128, 4, 2, 2, 256], BF16, name=f"mbt{j}")
                eng = nc.sync if j == 0 else nc.scalar
                eng.dma_start(
                    out=t, in_=mbd[b0 + j].rearrange(
                        "c mc h2 p n -> p c mc h2 n"))
                mbt[j] = t
            for fn in carry:
                fn()
            carry.clear()

            # ---- projections: fp8 DoubleRow ----
            qt = work.tile([128, 4, SBATCH, 256], BF16, name="qt")
            kt = work.tile([128, 4, SBATCH, 256], BF16, name="kt")
            for c in range(4):
                ps = psum.tile([128, NW], F32, tag="g", bufs=2)
                for i in range(2):
                    mm(ps, w8["wq"][:, 2 * i:2 * i + 2, c * 128:(c + 1) * 128],
                       xt8[:, 2 * i:2 * i + 2], start=i == 0, stop=i == 1,
                       perf_mode=DR)
                nc.scalar.copy(
                    out=qt[:, c].rearrange("p j n -> p (j n)"), in_=ps)
            for c in range(4):
                ps = psum.tile([128, NW], F32, tag="g", bufs=2)
                for i in range(2):
                    mm(ps, w8["wk"][:, 2 * i:2 * i + 2, c * 128:(c + 1) * 128],
                       ct8[:, 2 * i:2 * i + 2], start=i == 0, stop=i == 1,
                       perf_mode=DR)
                nc.scalar.copy(
                    out=kt[:, c].rearrange("p j n -> p (j n)"), in_=ps)
            # v token-major
            vt = work.tile([128, SBATCH, 2, 512], BF16, name="vt")
            for j in range(SBATCH):
                for mc in range(2):
                    ps = psum.tile([128, 512], F32, tag="g", bufs=2)
                    for i in range(2):
                        mm(ps, ct8[:, 2 * i:2 * i + 2, j, mc * 128:(mc + 1) * 128],
                           w8["wv"][:, 2 * i:2 * i + 2], start=i == 0, stop=i == 1,
                           perf_mode=DR)
                    nc.vector.tensor_copy(out=vt[:, j, mc, :], in_=ps)

            for fn in carry2:
                fn()
            carry2.clear()

            # ---- context branch: h = c @ Wc1 + bc1, 3-group fp8 Dekker ----
            # h*2^kc1 = c8@(W1A+W1B) + (32*dc)@W1C   (W1C = Wc1*2^kc1/32)
            ht = work.tile([128, 4, NW], BF16, name="ht")
            sqt = work.tile([128, 4, NW], F8, name="sqt")
            for c in range(4):
                ps = psum.tile([128, NW], F32, tag="g", bufs=2)
                cs = slice(c * 128, (c + 1) * 128)
                for i in range(2):
                    mm(ps, w8["w1a"][:, 2 * i:2 * i + 2, cs],
                       ct8[:, 2 * i:2 * i + 2], start=i == 0, stop=False,
                       perf_mode=DR)
                for i in range(2):
                    mm(ps, w8["w1b"][:, 2 * i:2 * i + 2, cs],
                       ct8[:, 2 * i:2 * i + 2], start=False, stop=False,
                       perf_mode=DR)
                for i in range(2):
                    mm(ps, w8["w1c"][:, 2 * i:2 * i + 2, cs],
                       clo[:, 2 * i:2 * i + 2], start=False, stop=i == 1,
                       perf_mode=DR)
                nc.scalar.activation(out=ht[:, c], in_=ps, func=AF.Identity,
                                     scale=2.0 ** (-kc1),
                                     bias=bc1c[:, c:c + 1])
                nc.gpsimd.tensor_mul(out=sqt[:, c], in0=ht[:, c], in1=ht[:, c])

            # ---- LN stats (closures; interleaved into attention) ----
            state = {}

            def ln_mu():
                # mu_sum via Wc1 row-sum weights applied to the fp8 context
                mu_ps = psum.tile([128, NW], F32, tag="g", bufs=2)
                for kc in range(4):
                    mm(mu_ps[0:1, :], w1st[:, kc, :],
                       ct8[:, kc].rearrange("p j n -> p (j n)"),
                       start=kc == 0, stop=kc == 3)
                mu_r = rows.tile([1, NW], F32, tag="r", bufs=6)
                nc.scalar.copy(out=mu_r, in_=mu_ps[0:1, :])
                state["mu_r"] = mu_r

            def ln_var():
                sq_ps = psum.tile([128, NW], F32, tag="g", bufs=2)
                for kc in range(4):
                    mm(sq_ps[0:1, :], colones, sqt[:, kc],
                       start=kc == 0, stop=kc == 3)
                mu_r = state["mu_r"]
                ms_r = rows.tile([1, NW], F32, tag="r", bufs=6)
                nc.vector.scalar_tensor_tensor(
                    out=ms_r, in0=mu_r, scalar=1.0 / DIM,
                    in1=mu_r, op0=ALU.mult, op1=ALU.mult)
                var_r = rows.tile([1, NW], F32, tag="r", bufs=6)
                nc.vector.tensor_sub(out=var_r, in0=sq_ps[0:1, :], in1=ms_r)
                ln_r = rows.tile([1, NW], F32, tag="r", bufs=6)
                nc.scalar.activation(out=ln_r, in_=var_r, func=AF.Ln,
                                     bias=eps512)
                a_r = rows.tile([1, NW], BF16, tag="r", bufs=6)
                nc.scalar.activation(out=a_r, in_=ln_r, func=AF.Exp,
                                     scale=-0.5)
                d_r = rows.tile([1, NW], BF16, tag="r", bufs=6)
                nc.vector.scalar_tensor_tensor(
                    out=d_r, in0=mu_r, scalar=-1.0 / DIM,
                    in1=a_r, op0=ALU.mult, op1=ALU.mult)
                ad_ps = psum.tile([128, 2, NW], F32, tag="s", bufs=2)
                mm(ad_ps[:, 0, :], onecol, a_r, start=True, stop=True)
                mm(ad_ps[:, 1, :], onecol, d_r, start=True, stop=True)
                ad_sb = work.tile([128, 2, NW], BF16, name="ad_sb")
                nc.vector.tensor_copy(out=ad_sb, in_=ad_ps)
                state["ad_sb"] = ad_sb

            # ---- attention + interleaved LN-normalize / ctx2+out1 ----
            ot = work.tile([128, 4, SBATCH, 256], F8, name="ot")
            res = iop.tile([128, 4, SBATCH, 256], BF16, name="res")

            def attn1(c, j):
                s_ps = psum.tile([128, 2, NW], F32, tag="s", bufs=2)
                for h2 in range(2):
                    p0 = 64 * h2
                    for mc in range(2):
                        mm(s_ps[:, h2, mc * 256:(mc + 1) * 256],
                           kt[p0:p0 + 64, c, j, mc * 128:(mc + 1) * 128],
                           qt[p0:p0 + 64, c, j], start=True, stop=True)
                pt = ppool.tile([128, 2, 2, 256], BF16, tag="p", name="pt",
                                bufs=6)
                nc.scalar.activation(
                    out=pt.rearrange("p mc h2 n -> p h2 mc n"),
                    in_=s_ps.rearrange("p h2 (mc n) -> p h2 mc n", mc=2),
                    func=AF.Exp, scale=2.0 ** (-(kq + kk)))
                nc.vector.tensor_mul(out=pt, in0=pt, in1=mbt[j][:, c])
                return s_ps, pt

            def attn2(c, j, s_ps, pt):
                # reuse the scores tile's first bank for the denominator
                sbc = s_ps[:, 0, :]
                for mc in range(2):
                    mm(sbc, ones128,
                       pt[:, mc].rearrange("p h n -> p (h n)"),
                       start=mc == 0, stop=mc == 1)
                oo = psum.tile([64, 2, 256], F32, tag="oo", bufs=2)
                for h2 in range(2):
                    hd = (2 * c + h2) * 64
                    for mc in range(2):
                        mm(oo[:, h2, :], vt[:, j, mc, hd:hd + 64],
                           pt[:, mc, h2, :], start=mc == 0, stop=mc == 1)
                rec_sb = ppool.tile([128, NW], F32, tag="rb", name="rec_sb",
                                    bufs=4)
                nc.vector.reciprocal(out=rec_sb, in_=sbc)
                for h2 in range(2):
                    nc.vector.tensor_mul(
                        out=ot[h2 * 64:(h2 + 1) * 64, c, j],
                        in0=oo[:, h2, :],
                        in1=rec_sb[h2 * 64:(h2 + 1) * 64,
                                   h2 * 256:(h2 + 1) * 256])

            def normalize(c):
                # rl = relu(((h*a + d)) * (g*sqrt(512)) + b), in place
                ad_sb = state["ad_sb"]
                nc.gpsimd.tensor_mul(out=ht[:, c], in0=ht[:, c],
                                     in1=ad_sb[:, 0, :])
                nc.gpsimd.tensor_add(out=ht[:, c], in0=ht[:, c],
                                     in1=ad_sb[:, 1, :])
                nc.scalar.activation(out=ht[:, c], in_=ht[:, c],
                                     func=AF.Relu,
                                     scale=lngc[:, c:c + 1],
                                     bias=lnbc[:, c:c + 1])

            def ctx2wo(j, ht=ht, ot=ot, res=res, b0=b0):
                co = psum.tile([128, 2, NW], F32, tag="s", bufs=2)
                for f in range(4):
                    dst = co[:, f // 2, (f % 2) * 256:(f % 2) * 256 + 256]
                    for kc in range(4):
                        mm(dst, wc2t[:, kc, f * 128:(f + 1) * 128],
                           ht[:, kc, j * 256:(j + 1) * 256],
                           start=kc == 0, stop=False)
                    for i in range(2):
                        mm(dst,
                           w8["wo"][:, 2 * i:2 * i + 2, f * 128:(f + 1) * 128],
                           ot[:, 2 * i:2 * i + 2, j, :],
                           start=False, stop=i == 1, perf_mode=DR)
                for f in range(4):
                    nc.scalar.activation(
                        out=res[:, f, j, :],
                        in_=co[:, f // 2, (f % 2) * 256:(f % 2) * 256 + 256],
                        func=AF.Identity, scale=2.0 ** (-g),
                        bias=bocbc[:, f:f + 1])
                carry.append(lambda j=j, res=res, b0=b0: nc.sync.dma_start(
                    out=outT[b0 + j].rearrange("(c p) n -> p c n", p=128),
                    in_=res[:, :, j, :]))

            # software-pipelined emission: stage2(n) always comes after
            # stage1(n+1) so no engine queue blocks on a not-yet-ready op
            p00 = attn1(0, 0)
            p10 = attn1(1, 0)
            attn2(0, 0, *p00)
            ln_mu()
            p20 = attn1(2, 0)
            attn2(1, 0, *p10)
            ln_var()
            p30 = attn1(3, 0)
            attn2(2, 0, *p20)
            normalize(0)
            normalize(1)
            p01 = attn1(0, 1)
            attn2(3, 0, *p30)
            normalize(2)
            normalize(3)
            p11 = attn1(1, 1)
            attn2(0, 1, *p01)
            ctx2wo(0)
            p21 = attn1(2, 1)
            attn2(1, 1, *p11)
            p31 = attn1(3, 1)
            attn2(2, 1, *p21)
            attn2(3, 1, *p31)
            carry2.append(lambda f=ctx2wo: f(1))
        for fn in carry2:
            fn()
        for fn in carry:
            fn()


def build(n_super, ks):
    # Pin the activation table: expose only natural_log_exp_and_others
    # (contains Exp/Ln/Relu/Identity/Copy/Square) to the act-table-load
    # insertion pass so it never flip-flops between sets.  Other entries are
    # emptied (not removed) to keep act_func_set_id indices valid.
    import concourse.bacc as bacc_mod
    from concourse.hw_specs import get_activation_tables as _gat

    def pinned_tables(arch):
        tabs = _gat(arch)
        return {name: (s if name == "natural_log_exp_and_others" else set())
                for name, s in tabs.items()}

    nc = bacc.Bacc("TRN2", target_bir_lowering=False, debug=False,
                   num_devices=N_CORES)
    dt = nc.dram_tensor
    io = (
        dt("x8", [NSUPER, 128, 4, SBATCH, N], F8,
           kind="ExternalInput").ap(),
        dt("c8", [NSUPER, 128, 4, SBATCH, N], F8,
           kind="ExternalInput").ap(),
        dt("clo", [NSUPER, 128, 4, SBATCH, N], F8,
           kind="ExternalInput").ap(),
        dt("mb", [BPC, 4, 2, 2, 128, N], BF16, kind="ExternalInput").ap(),
        dt("wq", [DIM, DIM], F8, kind="ExternalInput").ap(),
        dt("wk", [DIM, DIM], F8, kind="ExternalInput").ap(),
        dt("wv", [DIM, DIM], F8, kind="ExternalInput").ap(),
        dt("wo", [DIM, DIM], F8, kind="ExternalInput").ap(),
        dt("w1a", [DIM, DIM], F8, kind="ExternalInput").ap(),
        dt("w1b", [DIM, DIM], F8, kind="ExternalInput").ap(),
        dt("w1c", [DIM, DIM], F8, kind="ExternalInput").ap(),
        dt("w1s", [DIM, 1], BF16, kind="ExternalInput").ap(),
        dt("wc2", [DIM, DIM], BF16, kind="ExternalInput").ap(),
        dt("lng", [DIM], F32, kind="ExternalInput").ap(),
        dt("lnb", [DIM], F32, kind="ExternalInput").ap(),
        dt("bc1", [DIM], F32, kind="ExternalInput").ap(),
        dt("bocb", [DIM], F32, kind="ExternalInput").ap(),
        dt("outT", [BPC, DIM, N], BF16, kind="ExternalOutput").ap(),
    )
    with tile.TileContext(nc) as tc:
        _emit(nc, tc, io, n_super, ks)
    saved = bacc_mod.get_activation_tables
    bacc_mod.get_activation_tables = pinned_tables
    try:
        nc.compile()
    finally:
        bacc_mod.get_activation_tables = saved
    return nc


def _k_of(absmax):
    return int(math.floor(math.log2(120.0 / max(absmax, 1e-30))))


def prep_in_maps(x, context, mask, Wq, Wk, Wv, Wc1, bc1, ln_g, ln_b, Wc2, bc2,
                 Wo, bo, bias_table, rel_index):
    f = np.float32
    x = np.asarray(x, f)
    context = np.asarray(context, f)
    mask = np.asarray(mask)
    Wq = np.asarray(Wq, f) * SCALE
    Wk = np.asarray(Wk, f)
    Wv = np.asarray(Wv, f)
    Wo = np.asarray(Wo, f)
    Wc1 = np.asarray(Wc1, f)
    Wc2 = np.asarray(Wc2, f)

    kq = _k_of(np.abs(Wq).max())
    kk = _k_of(np.abs(Wk).max())
    kv = _k_of(np.abs(Wv).max())
    kwo = _k_of(np.abs(Wo).max())
    kc1 = _k_of(np.abs(Wc1).max())
    w1sum = Wc1.sum(axis=1, keepdims=True)
    ksum = 0
    lng_a = np.asarray(ln_g, f)
    lnb_a = np.asarray(ln_b, f)
    guni = (float(lng_a[0]) if np.all(lng_a == lng_a[0]) and float(lng_a[0]) > 0
            and np.all(lnb_a == 0.0) else None)
    ks = (kq, kk, kv, kwo, kc1, ksum, guni)
    g = OT_K + kwo

    xT = np.ascontiguousarray(
        x.reshape(N_CORES, BPC, N, DIM).transpose(0, 1, 3, 2))
    cT = np.ascontiguousarray(
        context.reshape(N_CORES, BPC, N, DIM).transpose(0, 1, 3, 2))
    x8 = xT.astype(NP8)
    c8 = cT.astype(NP8)
    clo = ((cT - c8.astype(f)) * 32.0).astype(NP8)

    def slab(a):
        # [cr, BPC, 512, 256] -> [cr, NSUPER, 128(p), 4(kc), SBATCH(j), 256]
        a = a.reshape(N_CORES, NSUPER, SBATCH, 4, 128, N)
        return np.ascontiguousarray(a.transpose(0, 1, 4, 3, 2, 5))

    x8 = slab(x8)
    c8 = slab(c8)
    clo = slab(clo)

    # mb[core, b, c, mc, h2, p, n] = maskT[b, m, n] * exp(bias)[h, m, n]
    expBT = np.exp(
        np.asarray(bias_table, f)[np.asarray(rel_index)].transpose(2, 1, 0))
    mT = mask.reshape(N_CORES, BPC, N, N).transpose(0, 1, 3, 2).astype(f)
    mbf = mT[:, :, None, :, :] * expBT[None, None, :, :, :]  # [cr,b,h,m,n]
    mbf = mbf.reshape(N_CORES, BPC, 4, 2, 2, 128, N).transpose(
        0, 1, 2, 4, 3, 5, 6)  # [cr, b, c, mc, h2, p, n]
    mb = np.ascontiguousarray(mbf).astype(NPBF)

    w1as = Wc1 * 2.0 ** kc1
    w1a = w1as.astype(NP8)
    w1b = (w1as - w1a.astype(f)).astype(NP8)
    w1c = (w1as / 32.0).astype(NP8)

    shared = dict(
        wq=np.ascontiguousarray(Wq * 2.0 ** kq).astype(NP8),
        wk=np.ascontiguousarray(Wk * 2.0 ** kk).astype(NP8),
        wv=np.ascontiguousarray(Wv * 2.0 ** kv).astype(NP8),
        wo=np.ascontiguousarray(Wo * 2.0 ** kwo).astype(NP8),
        w1a=np.ascontiguousarray(w1a),
        w1b=np.ascontiguousarray(w1b),
        w1c=np.ascontiguousarray(w1c),
        w1s=np.ascontiguousarray(w1sum).astype(NPBF),
        wc2=np.ascontiguousarray(Wc2 * 2.0 ** g).astype(NPBF),
        lng=np.ascontiguousarray(np.asarray(ln_g, f) * math.sqrt(DIM)),
        lnb=np.ascontiguousarray(np.asarray(ln_b, f)),
        bc1=np.ascontiguousarray(np.asarray(bc1, f)),
        bocb=np.ascontiguousarray(np.asarray(bo, f) + np.asarray(bc2, f)),
    )
    in_maps = [dict(x8=x8[c], c8=c8[c], clo=clo[c], mb=mb[c], **shared)
               for c in range(N_CORES)]
    return in_maps, ks


_nc_cache = {}


def _get_nc(n_super, ks):
    key = (n_super, ks)
    if key not in _nc_cache:
        _nc_cache[key] = build(n_super, ks)
    return _nc_cache[key]


def assemble_out(results):
    outT = np.stack([np.asarray(results[c]["outT"]).astype(np.float32)
                     for c in range(N_CORES)])
    return np.ascontiguousarray(
        outT.transpose(0, 1, 3, 2).reshape(B, N, DIM))


def kernel(**inputs):
    in_maps, ks = prep_in_maps(**inputs)
    nc = _get_nc(NSUPER, ks)
    res = run_bass_kernel_spmd(nc, in_maps, core_ids=list(range(N_CORES)))
    return assemble_out(res.results)
